# revision 1
# baseline (speedup 1.0000x reference)
"""Trainium2 Bass kernel for nn_AttentionKernelIntegral (linear attention
with instance-normed k/v, collapsed algebraically).

Math
----
Reference computes (per batch, H=8 heads, D=64, C=OUT=256, N=16384):
    q = u @ Wq^T ; k = u @ Wk^T ; v = u @ Wv^T          (per head blocks)
    khat = instnorm_n(k); vhat = instnorm_n(v)
    kv_h = (1/N) khat_h^T vhat_h                        [D, D]
    out  = concat_h(q_h @ kv_h) @ Wo^T + bo

Everything downstream of u is linear except the instance-norm statistics
(exact functions of first/second moments over n), so the network
collapses to   out = u @ W_eff + bo   with

    Cuu   = u^T u,  su = u^T 1                          [C, C], [C]
    mk    = Wk su / N ;  var_k = diag(Wk Cuu Wk^T)/N - mk^2
    rk    = rsqrt(var_k + eps)          (mv, var_v, rv likewise)
    kvT_h = rk_d rv_e ((Wv Cuu Wk^T)_h/N - mv_h mk_h^T)     [e, d]
    W_eff = sum_h Wq_h^T kvT_h^T Wo_h^T                 [C, OUT]

Sharding: 8 cores = 4 batches x 2 grid-halves.  Each core receives the
full u for its batch with ITS half permuted first (Cuu spans the full
grid; cheaper than a pairwise AllReduce, which measures ~29us), and
emits the output for its own half.  Host reassembles halves + bo.

dtypes: all O(N) PE work in bf16 (FWL weight loads, 1 cyc/row); Cuu is
accumulated in fp32 PSUM; scalar statistics math in fp32, with rsqrt
done in column format (128 partitions) via 1-contraction matmuls.

Layout: row tiles are loaded interleaved ("(p j) c -> p j c") so each
DMA descriptor moves 16KB contiguous per partition (2MB per DMA);
input and output use the same interleave so it cancels.
"""

import numpy as np

import concourse.bass as bass
import concourse.tile as tile
from concourse import bacc, mybir
from concourse.bass_utils import run_bass_kernel_spmd
from concourse.masks import make_identity

F32 = mybir.dt.float32
BF16 = mybir.dt.bfloat16
AL = mybir.AluOpType
AF = mybir.ActivationFunctionType

P = 128
N_FULL = 16384
N_HALF = 8192
C = 256
HD = 512          # H * D
OUT = 256
EPS = 1e-5
CH_ROWS = 2048
N_CHUNKS = N_FULL // CH_ROWS      # 8 chunks of 2048 rows (full grid)
SUBT = CH_ROWS // P               # 16 row-subtiles per chunk
G_ALL = N_FULL // P               # 128 row-tiles total
G_MINE = N_HALF // P              # first 64 belong to this core
ON_CHUNKS = N_HALF // CH_ROWS     # output chunks (same interleave)
INV_N = 1.0 / float(N_FULL)


def build_nc():
    nc = bacc.Bacc(
        "TRN2",
        target_bir_lowering=False,
        debug=False,
        num_devices=8,
    )
    u_d = nc.dram_tensor("u", [N_FULL, C], F32, kind="ExternalInput").ap()
    wq_d = nc.dram_tensor("wq", [HD, C], F32, kind="ExternalInput").ap()
    wk_d = nc.dram_tensor("wk", [HD, C], F32, kind="ExternalInput").ap()
    wv_d = nc.dram_tensor("wv", [HD, C], F32, kind="ExternalInput").ap()
    wo_d = nc.dram_tensor("wo", [OUT, HD], F32, kind="ExternalInput").ap()
    out_d = nc.dram_tensor("out", [N_HALF, OUT], F32, kind="ExternalOutput").ap()

    with tile.TileContext(nc) as tc:
        with tc.tile_pool(name="pers", bufs=1) as pers:
            # ---- persistent tiles -------------------------------------
            uT = pers.tile([P, 2, N_HALF], BF16)         # u^T (bf16)
            ident = pers.tile([P, P], F32)
            make_identity(nc, ident[:])
            ident_bf = pers.tile([P, P], BF16)
            nc.vector.tensor_copy(ident_bf[:], ident[:])
            wq_bf = pers.tile([P, 4, C], BF16)           # Wq natural [hd, c]
            wkT_bf = pers.tile([P, 2, HD], BF16)         # Wk^T  [c, hd]
            wvT_bf = pers.tile([P, 2, HD], BF16)
            woT_bf = pers.tile([P, 4, OUT], BF16)        # Wo^T  [hd, o]
            weff = pers.tile([P, 2, OUT], BF16)
            cuu = pers.tile([P, 2, C + 1], F32)
            cuu_bf = pers.tile([P, 2, C + 1], BF16)
            ones_col_f = pers.tile([P, 1], F32)
            nc.vector.memset(ones_col_f[:], 1.0)
            ones_row = pers.tile([1, P], F32)
            nc.vector.memset(ones_row[:], 1.0)
            one1 = pers.tile([1, 1], F32)
            nc.vector.memset(one1[:], 1.0)
            # prewarm ACT tables used later (Copy via scalar.mul, Sqrt)
            warm = pers.tile([1, 8], F32)
            nc.vector.memset(warm[:], 1.0)
            nc.scalar.mul(warm[:], warm[:], 1.0)
            nc.scalar.activation(warm[:], warm[:], AF.Sqrt)

            # ---- phase 1: weights prep, then stream u -----------------
            with (
                tc.tile_pool(name="upool", bufs=3) as upool,
                tc.tile_pool(name="pacc", bufs=1, space="PSUM") as pacc,
                tc.tile_pool(name="wstage", bufs=2) as wst,
                tc.tile_pool(name="wps", bufs=2, space="PSUM") as wps,
                tc.tile_pool(name="ptr", bufs=3, space="PSUM") as ptr,
            ):
                # weight prep: cast to bf16, transpose on PE (bf16)
                wq_f = wst.tile([P, 4, C], F32, tag="wnat", name="wq_f")
                nc.sync.dma_start(wq_f[:], wq_d.rearrange("(a p) c -> p a c", p=P))
                nc.vector.tensor_copy(wq_bf[:], wq_f[:])
                for w_d, wT_t in ((wk_d, wkT_bf), (wv_d, wvT_bf)):
                    wnat = wst.tile([P, 4, C], F32, tag="wnat", name="wnat")
                    nc.sync.dma_start(
                        wnat[:], w_d.rearrange("(a p) c -> p a c", p=P)
                    )
                    wnat_bf = wst.tile([P, 4, C], BF16, tag="wnbf", name="wnat_bf")
                    nc.vector.tensor_copy(wnat_bf[:], wnat[:])
                    for a in range(4):
                        for b2 in range(2):
                            pst = wps.tile([P, P], BF16, tag="wt", name="pst")
                            nc.tensor.transpose(
                                pst[:], wnat_bf[:, a, b2 * P:(b2 + 1) * P],
                                ident_bf[:],
                            )
                            nc.any.tensor_copy(
                                wT_t[:, b2, a * P:(a + 1) * P], pst[:]
                            )
                wonat = wst.tile([P, 2, HD], F32, tag="wonat", name="wonat")
                nc.sync.dma_start(
                    wonat[:], wo_d.rearrange("(a p) c -> p a c", p=P)
                )
                wonat_bf = wst.tile([P, 2, HD], BF16, tag="wnbf", name="wonat_bf")
                nc.vector.tensor_copy(wonat_bf[:], wonat[:])
                for a in range(2):
                    for b2 in range(4):
                        pst = wps.tile([P, P], BF16, tag="wt", name="pst")
                        nc.tensor.transpose(
                            pst[:], wonat_bf[:, a, b2 * P:(b2 + 1) * P],
                            ident_bf[:],
                        )
                        nc.any.tensor_copy(
                            woT_bf[:, b2, a * P:(a + 1) * P], pst[:]
                        )

                # Cuu (+su via ones column) accumulation; u^T transposes
                # for this core's half.  Chunk 0 arrives in three j-slices
                # (same 16-way row interleave) so the PE starts early.
                cps = [
                    pacc.tile([P, C + 1], F32, tag=f"c{t}", name=f"c{t}")
                    for t in range(2)
                ]
                sched = [(0, 0, 4), (0, 4, 4), (0, 8, 8)]
                for ch in range(1, N_CHUNKS):
                    sched.append((ch, 0, SUBT))
                u1 = None
                ubf = None
                for ch, j0, nsub in sched:
                    if j0 == 0:
                        u1 = upool.tile([P, SUBT, C + 1], F32, tag="u1", name="u1")
                        ubf = upool.tile(
                            [P, SUBT, C + 1], BF16, tag="ubf", name="ubf"
                        )
                    src_ap = u_d[ch * CH_ROWS:(ch + 1) * CH_ROWS, :].rearrange(
                        "(p j) c -> p j c", p=P
                    )
                    nc.sync.dma_start(
                        u1[:, j0:j0 + nsub, 0:C], src_ap[:, j0:j0 + nsub, :]
                    )
                    nc.vector.memset(u1[:, j0:j0 + nsub, C:C + 1], 1.0)
                    nc.vector.tensor_copy(
                        ubf[:, j0:j0 + nsub, :], u1[:, j0:j0 + nsub, :]
                    )
                    for j in range(j0, j0 + nsub):
                        g = ch * SUBT + j
                        for t in range(2):
                            nc.tensor.matmul(
                                cps[t][:],
                                ubf[:, j, t * P:(t + 1) * P],
                                ubf[:, j, :],
                                start=(g == 0),
                                stop=(g == G_ALL - 1),
                            )
                        if g < G_MINE:
                            tps = ptr.tile([P, 2 * P], BF16, tag="uT", name="tps")
                            for t in range(2):
                                nc.tensor.transpose(
                                    tps[:, t * P:(t + 1) * P],
                                    ubf[:, j, t * P:(t + 1) * P],
                                    ident_bf[:],
                                )
                            nc.vector.tensor_copy(
                                uT[:, :, g * P:(g + 1) * P],
                                tps[:].rearrange("p (t n) -> p t n", t=2),
                            )

                for t in range(2):
                    nc.any.tensor_copy(cuu[:, t, :], cps[t][:])
                nc.vector.tensor_copy(cuu_bf[:], cuu[:])

            # ---- phase 2: statistics / W_eff (tiny) -------------------
            with tc.tile_pool(name="sm", bufs=1) as sm:
                psA_ctx = tc.tile_pool(name="psA", bufs=1, space="PSUM")
                psA = psA_ctx.__enter__()
                a_k = sm.tile([P, 2, HD], BF16)
                m_k = sm.tile([P, 2, HD], F32)
                a_v = sm.tile([P, 2, HD], BF16)
                m_v = sm.tile([P, 2, HD], F32)
                # A_k = Cuu @ Wk^T, A_v = Cuu @ Wv^T   [c, hd]  (bf16)
                for wT_t, a_t, mm_t in ((wkT_bf, a_k, m_k), (wvT_bf, a_v, m_v)):
                    for t in range(2):
                        aps = psA.tile([P, HD], F32, tag="aps", bufs=2, name="aps")
                        for tp in range(2):
                            nc.tensor.matmul(
                                aps[:],
                                cuu_bf[:, tp, t * P:(t + 1) * P],
                                wT_t[:, tp, :],
                                start=(tp == 0),
                                stop=(tp == 1),
                            )
                        nc.vector.tensor_copy(a_t[:, t, :], aps[:])
                        # M = W^T .* A (second-moment integrand, fp32 out)
                        nc.vector.tensor_mul(mm_t[:, t, :], aps[:], wT_t[:, t, :])

                # first/second moment rows [1, HD]
                mk = sm.tile([1, HD], F32)
                mv = sm.tile([1, HD], F32)
                ekk = sm.tile([1, HD], F32)
                evv = sm.tile([1, HD], F32)
                tk = sm.tile([1, HD], F32)
                tv = sm.tile([1, HD], F32)
                vark = sm.tile([1, HD], F32)
                varv = sm.tile([1, HD], F32)
                for wT_t, m_t in ((wkT_bf, mk), (wvT_bf, mv)):
                    sps = psA.tile([1, HD], F32, tag="st", bufs=2, name="sps")
                    for tp in range(2):
                        nc.tensor.matmul(
                            sps[:],
                            cuu_bf[:, tp, C:C + 1],
                            wT_t[:, tp, :],
                            start=(tp == 0),
                            stop=(tp == 1),
                        )
                    nc.scalar.activation(m_t[:], sps[:], AF.Copy, scale=INV_N)
                for m_src, e_t in ((m_k, ekk), (m_v, evv)):
                    sps = psA.tile([1, HD], F32, tag="st", bufs=2, name="sps")
                    for tp in range(2):
                        nc.tensor.matmul(
                            sps[:],
                            ones_col_f[:],
                            m_src[:, tp, :],
                            start=(tp == 0),
                            stop=(tp == 1),
                        )
                    nc.scalar.activation(e_t[:], sps[:], AF.Copy, scale=INV_N)
                # var rows = e - m^2 (k and v chains independent)
                nc.vector.tensor_mul(tk[:], mk[:], mk[:])
                nc.vector.tensor_mul(tv[:], mv[:], mv[:])
                nc.vector.tensor_sub(vark[:], ekk[:], tk[:])
                nc.vector.tensor_sub(varv[:], evv[:], tv[:])

                # rsqrt in column format (fast across 128 partitions)
                eps_col = sm.tile([P, 4], F32)
                nc.vector.memset(eps_col[:], EPS)
                rk_col = sm.tile([P, 4], F32)
                rv_col = sm.tile([P, 4], F32)
                for var_row, r_col in ((vark, rk_col), (varv, rv_col)):
                    vc = psA.tile([P, 4], F32, tag="vc", bufs=2, name="vc")
                    for g in range(4):
                        nc.tensor.matmul(
                            vc[:, g:g + 1],
                            var_row[0:1, g * P:(g + 1) * P],
                            one1[:],
                            start=True,
                            stop=True,
                        )
                    nc.vector.tensor_add(r_col[:], vc[:], eps_col[:])
                    nc.scalar.activation(r_col[:], r_col[:], AF.Sqrt)
                    nc.vector.reciprocal(r_col[:], r_col[:])
                # rk back to a row, broadcast across partitions
                rk_row = sm.tile([1, HD], F32)
                rk_bc = sm.tile([P, HD], F32)
                rps = psA.tile([1, HD], F32, tag="st", bufs=2, name="rps")
                for g in range(4):
                    nc.tensor.matmul(
                        rps[0:1, g * P:(g + 1) * P],
                        rk_col[:, g:g + 1],
                        ident[:],
                        start=True,
                        stop=True,
                    )
                nc.scalar.mul(rk_row[:], rps[:], 1.0)
                bps = psA.tile([P, HD], F32, tag="aps", bufs=2, name="bps")
                nc.tensor.matmul(bps[:], ones_row[:], rk_row[:], start=True, stop=True)
                nc.any.tensor_copy(rk_bc[:], bps[:])
                psA_ctx.__exit__(None, None, None)

                # per head-pair: kvT = ((Wv Cuu Wk^T)/N - mv^T mk) .* rk (free)
                # .* rv (partition), then B = kvT^T-contract Wo^T, acc W_eff
                with tc.tile_pool(name="psP", bufs=1, space="PSUM") as psP:
                    wps2 = [
                        psP.tile([P, OUT], F32, tag=f"weff{t}", name=f"wps{t}")
                        for t in range(2)
                    ]
                    for jp in range(4):
                        sl = slice(jp * P, (jp + 1) * P)
                        sd = psP.tile([P, P], F32, tag="sd", bufs=2, name="sd")
                        for tp in range(2):
                            nc.tensor.matmul(
                                sd[:],
                                wvT_bf[:, tp, sl],
                                a_k[:, tp, sl],
                                start=(tp == 0),
                                stop=(tp == 1),
                            )
                        outr = psP.tile([P, P], F32, tag="outr", bufs=2, name="outr")
                        nc.tensor.matmul(
                            outr[:], mv[:, sl], mk[:, sl], start=True, stop=True
                        )
                        kvp = sm.tile([P, P], F32, tag=f"kv{jp}", name=f"kv{jp}")
                        nc.vector.memset(kvp[:], 0.0)
                        for g in range(2):
                            gs = slice(g * 64, g * 64 + 64)
                            nc.scalar.mul(kvp[gs, gs], sd[gs, gs], INV_N)
                            nc.vector.tensor_sub(
                                kvp[gs, gs], kvp[gs, gs], outr[gs, gs]
                            )
                        nc.vector.tensor_mul(kvp[:], kvp[:], rk_bc[:, sl])
                        kvp_bf = sm.tile([P, P], BF16, tag=f"kvb{jp}", name=f"kvb{jp}")
                        nc.vector.tensor_scalar_mul(
                            kvp_bf[:], kvp[:], rv_col[:, jp:jp + 1]
                        )
                        bps2 = psP.tile([P, OUT], F32, tag="bps2", bufs=2, name="bps2")
                        nc.tensor.matmul(
                            bps2[:], kvp_bf[:], woT_bf[:, jp, :], start=True, stop=True
                        )
                        bsb = sm.tile([P, OUT], BF16, tag="bsb", name="bsb")
                        nc.any.tensor_copy(bsb[:], bps2[:])
                        for t in range(2):
                            nc.tensor.matmul(
                                wps2[t][:],
                                wq_bf[:, jp, t * P:(t + 1) * P],
                                bsb[:],
                                start=(jp == 0),
                                stop=(jp == 3),
                            )
                    for t in range(2):
                        nc.any.tensor_copy(weff[:, t, :], wps2[t][:])

            # ---- phase 3: out = u @ W_eff (bf16 PE stream) ------------
            with (
                tc.tile_pool(name="opool", bufs=2) as opool,
                tc.tile_pool(name="pout", bufs=6, space="PSUM") as pout,
            ):
                for ch in range(ON_CHUNKS):
                    osb = opool.tile([P, SUBT, OUT], F32, tag="osb", name="osb")
                    for j in range(SUBT):
                        g = ch * SUBT + j
                        ops = pout.tile([P, OUT], F32, tag="ops", name="ops")
                        for t in range(2):
                            nc.tensor.matmul(
                                ops[:],
                                uT[:, t, g * P:(g + 1) * P],
                                weff[:, t, :],
                                start=(t == 0),
                                stop=(t == 1),
                            )
                        if j % 2 == 0:
                            nc.vector.tensor_copy(osb[:, j, :], ops[:])
                        else:
                            nc.scalar.mul(osb[:, j, :], ops[:], 1.0)
                    nc.sync.dma_start(
                        out_d[ch * CH_ROWS:(ch + 1) * CH_ROWS, :].rearrange(
                            "(p j) c -> p j c", p=P
                        ),
                        osb[:],
                    )

    nc.compile()
    return nc


_NC_CACHE = None


def _get_nc():
    global _NC_CACHE
    if _NC_CACHE is None:
        _NC_CACHE = build_nc()
    return _NC_CACHE


def make_in_maps(u_src, Wq, Wk, Wv, Wo):
    """Per-core input dicts. Core c = (batch c//2, half c%2); its own
    half of the grid axis is permuted to the front of u."""
    in_maps = []
    for c in range(8):
        b, half = c // 2, c % 2
        ub = u_src[b]
        mine = ub[half * N_HALF:(half + 1) * N_HALF]
        other = ub[(1 - half) * N_HALF:(2 - half) * N_HALF]
        u_perm = np.ascontiguousarray(np.concatenate([mine, other], axis=0))
        in_maps.append(
            {
                "u": u_perm,
                "wq": np.ascontiguousarray(Wq),
                "wk": np.ascontiguousarray(Wk),
                "wv": np.ascontiguousarray(Wv),
                "wo": np.ascontiguousarray(Wo),
            }
        )
    return in_maps


def assemble_output(results, bo):
    out = np.empty((4, N_FULL, OUT), dtype=np.float32)
    for c in range(8):
        b, half = c // 2, c % 2
        out[b, half * N_HALF:(half + 1) * N_HALF] = results[c]["out"]
    if np.any(bo):
        out += bo.reshape(1, 1, OUT)
    return out


def run(inputs, trace=False, tmpdir=None):
    """inputs: dict as from reference.setup_inputs(). Returns
    (full_output, BassKernelResults)."""
    u_src = np.asarray(inputs["u_src"], dtype=np.float32)
    Wq = np.asarray(inputs["Wq"], dtype=np.float32)
    Wk = np.asarray(inputs["Wk"], dtype=np.float32)
    Wv = np.asarray(inputs["Wv"], dtype=np.float32)
    Wo = np.asarray(inputs["Wo"], dtype=np.float32)
    bo = np.asarray(inputs["bo"], dtype=np.float32)
    nc = _get_nc()
    in_maps = make_in_maps(u_src, Wq, Wk, Wv, Wo)
    res = run_bass_kernel_spmd(
        nc, in_maps, core_ids=list(range(8)), trace=trace, tmpdir=tmpdir
    )
    return assemble_output(res.results, bo), res


def kernel(**inputs):
    out, _ = run(inputs, trace=False)
    return out



# revision 9
# speedup vs baseline: 1.4831x; 1.4831x over previous
"""Trainium2 Bass kernel for nn_AttentionKernelIntegral (linear attention
with instance-normed k/v, collapsed algebraically).

Math
----
Reference computes (per batch, H=8 heads, D=64, C=OUT=256, N=16384):
    q = u @ Wq^T ; k = u @ Wk^T ; v = u @ Wv^T          (per head blocks)
    khat = instnorm_n(k); vhat = instnorm_n(v)
    kv_h = (1/N) khat_h^T vhat_h                        [D, D]
    out  = concat_h(q_h @ kv_h) @ Wo^T + bo

Everything downstream of u is linear except the instance-norm statistics
(exact functions of first/second moments over n), so the network
collapses to   out = u @ W_eff + bo.  With the *centered* covariance

    Ctilde = Cuu/N - (su/N)(su/N)^T,   Cuu = u^T u, su = u^T 1

the means drop out entirely:

    kv_h   = Dk_h (Wk_h Ctilde Wv_h^T) Dv_h
    vark_d = (Wk Ctilde Wk^T)_dd ;  Dk = diag(rsqrt(vark + eps))
    W_eff  = sum_h Wq_h^T kv_h Wo_h^T                   [C, OUT]

Sharding: 8 cores = 4 batches x 2 grid-halves.  Each core receives the
full u for its batch (bf16, with ITS half permuted first), accumulates
Cuu over the full grid, and emits out^T for its own half.

dtypes: u/weights are cast to bf16 on the host (halves all HBM
traffic); PE accumulation is fp32 PSUM; the output is stored bf16
(o-major, out^T) and upcast + unpermuted on the host.

Cuu uses symmetry: the row-block-1 matmul streams only cols 128..256;
the missing [128,128] block of Ctilde is reconstructed by one PE
transpose.  Variances are produced directly in column format (N=1
matmuls against a ones column), and the rsqrt scales are folded into
Wq / Wo^T rows so the per-head-pair kv block needs only a block-diag
mask multiply.
"""

import numpy as np
import ml_dtypes

import concourse.bass as bass
import concourse.tile as tile
from concourse import bacc, mybir
from concourse.bass_utils import run_bass_kernel_spmd
from concourse.masks import make_identity, make_block_diagonal

F32 = mybir.dt.float32
BF16 = mybir.dt.bfloat16
AL = mybir.AluOpType
AF = mybir.ActivationFunctionType

P = 128
N_FULL = 16384
N_HALF = 8192
C = 256
HD = 512          # H * D
OUT = 256
EPS = 1e-5
CH_ROWS = 2048
N_CHUNKS = N_FULL // CH_ROWS      # 8 chunks of 2048 rows (full grid)
SUBT = CH_ROWS // P               # 16 row-subtiles per chunk
G_ALL = N_FULL // P               # 128 row-tiles total
G_MINE = N_HALF // P              # first 64 belong to this core
INV_N = 1.0 / float(N_FULL)
GROUP = 512                       # phase-3 row group (columns of out^T)
NGROUPS = N_HALF // GROUP         # 16


def build_nc():
    nc = bacc.Bacc(
        "TRN2",
        target_bir_lowering=False,
        debug=False,
        num_devices=8,
    )
    u_d = nc.dram_tensor("u", [N_FULL, C], BF16, kind="ExternalInput").ap()
    wq_d = nc.dram_tensor("wq", [HD, C], BF16, kind="ExternalInput").ap()
    wkt_d = nc.dram_tensor("wkt", [C, HD], BF16, kind="ExternalInput").ap()
    wvt_d = nc.dram_tensor("wvt", [C, HD], BF16, kind="ExternalInput").ap()
    wot_d = nc.dram_tensor("wot", [HD, OUT], BF16, kind="ExternalInput").ap()
    out_d = nc.dram_tensor("out", [OUT, N_HALF], BF16, kind="ExternalOutput").ap()

    with tile.TileContext(nc) as tc:
        with tc.tile_pool(name="pers", bufs=1) as pers:
            # ---- persistent tiles -------------------------------------
            uT = pers.tile([P, 2, N_HALF], BF16)         # u^T (bf16, own half)
            ident = pers.tile([P, P], F32)
            make_identity(nc, ident[:])
            ident_bf = pers.tile([P, P], BF16)
            nc.vector.tensor_copy(ident_bf[:], ident[:])
            mask_f = pers.tile([P, P], F32)
            make_block_diagonal(nc, mask_f[:], 64)
            mask_bf = pers.tile([P, P], BF16)
            nc.vector.tensor_copy(mask_bf[:], mask_f[:])
            wq_bf = pers.tile([P, 4, C], BF16)           # Wq natural [hd, c]
            wkT_bf = pers.tile([P, 2, HD], BF16)         # Wk^T  [c, hd]
            wvT_bf = pers.tile([P, 2, HD], BF16)
            woT_bf = pers.tile([P, 4, OUT], BF16)        # Wo^T  [hd, o]
            wq_s = pers.tile([P, 4, C], BF16)            # rk-scaled Wq
            wot_s = pers.tile([P, 4, OUT], BF16)         # rv-scaled Wo^T
            weff = pers.tile([P, 2, OUT], BF16)
            ct_bf = pers.tile([P, 2, C], BF16)           # Ctilde (bf16)
            ones_bf = pers.tile([P, 1], BF16)
            nc.vector.memset(ones_bf[:], 1.0)
            eps_col = pers.tile([P, 1], F32)
            nc.vector.memset(eps_col[:], EPS)
            su_col = pers.tile([P, 2], BF16)
            su_row = pers.tile([1, C], BF16)
            su_nrow = pers.tile([1, C], BF16)            # -su / N^2
            rk_col = pers.tile([P, 4], F32)
            rv_col = pers.tile([P, 4], F32)
            # prewarm ACT tables used later (Copy via scalar.mul, Sqrt)
            warm = pers.tile([1, 8], F32)
            nc.vector.memset(warm[:], 1.0)
            nc.scalar.mul(warm[:], warm[:], 1.0)
            nc.scalar.activation(warm[:], warm[:], AF.Sqrt)

            # ---- phase 1: stream u, accumulate Cuu, transpose own half
            with (
                tc.tile_pool(name="upool", bufs=3) as upool,
                tc.tile_pool(name="pacc", bufs=1, space="PSUM") as pacc,
                tc.tile_pool(name="ptr", bufs=3, space="PSUM") as ptr,
            ):
                cps0 = pacc.tile([P, C + 1], F32, tag="c0", name="c0")
                cps1 = pacc.tile([P, C + 1 - P], F32, tag="c1", name="c1")
                # chunk 0 arrives in three j-slices so the PE starts early
                sched = [(0, 0, 4), (0, 4, 4), (0, 8, 8)]
                for ch in range(1, N_CHUNKS):
                    sched.append((ch, 0, SUBT))
                ub = None
                first_dmas = 0
                for ch, j0, nsub in sched:
                    if j0 == 0:
                        ub = upool.tile([P, SUBT, C + 1], BF16, tag="ub", name="ub")
                    src_ap = u_d[ch * CH_ROWS:(ch + 1) * CH_ROWS, :].rearrange(
                        "(p j) c -> p j c", p=P
                    )
                    nc.sync.dma_start(
                        ub[:, j0:j0 + nsub, 0:C], src_ap[:, j0:j0 + nsub, :]
                    )
                    nc.vector.memset(ub[:, j0:j0 + nsub, C:C + 1], 1.0)
                    if first_dmas == 0:
                        # weights: issued after the first u slice is queued
                        nc.sync.dma_start(
                            wq_bf[:], wq_d.rearrange("(a p) c -> p a c", p=P)
                        )
                        nc.sync.dma_start(
                            wkT_bf[:], wkt_d.rearrange("(a p) c -> p a c", p=P)
                        )
                        nc.sync.dma_start(
                            wvT_bf[:], wvt_d.rearrange("(a p) c -> p a c", p=P)
                        )
                        nc.sync.dma_start(
                            woT_bf[:], wot_d.rearrange("(a p) c -> p a c", p=P)
                        )
                        first_dmas = 1
                    for j in range(j0, j0 + nsub):
                        g = ch * SUBT + j
                        nc.tensor.matmul(
                            cps0[:],
                            ub[:, j, 0:P],
                            ub[:, j, :],
                            start=(g == 0),
                            stop=(g == G_ALL - 1),
                        )
                        nc.tensor.matmul(
                            cps1[:],
                            ub[:, j, P:C],
                            ub[:, j, P:C + 1],
                            start=(g == 0),
                            stop=(g == G_ALL - 1),
                        )
                        if g < G_MINE:
                            tps = ptr.tile([P, C], BF16, tag="uT", name="tps")
                            for t in range(2):
                                nc.tensor.transpose(
                                    tps[:, t * P:(t + 1) * P],
                                    ub[:, j, t * P:(t + 1) * P],
                                    ident_bf[:],
                                )
                            if g % 2 == 0:
                                nc.vector.tensor_copy(
                                    uT[:, :, g * P:(g + 1) * P],
                                    tps[:].rearrange("p (t n) -> p t n", t=2),
                                )
                            else:
                                nc.scalar.copy(
                                    uT[:, :, g * P:(g + 1) * P],
                                    tps[:].rearrange("p (t n) -> p t n", t=2),
                                )

                # ---- Ctilde from cps + su (still holding cps psum) ----
                with tc.tile_pool(name="psm", bufs=1, space="PSUM") as psm:
                    # su columns -> bf16 sbuf
                    nc.scalar.copy(su_col[:, 0:1], cps0[:, C:C + 1])
                    nc.scalar.copy(su_col[:, 1:2], cps1[:, C - P:C - P + 1])
                    su_rowT = psm.tile([1, C], F32, tag="surt", name="surt")
                    for t in range(2):
                        nc.tensor.matmul(
                            su_rowT[0:1, t * P:(t + 1) * P],
                            su_col[:, t:t + 1],
                            ident_bf[:],
                            start=True,
                            stop=True,
                        )
                    nc.vector.tensor_copy(su_row[:], su_rowT[:])
                    nc.scalar.activation(
                        su_nrow[:], su_rowT[:], AF.Copy, scale=-INV_N
                    )
                    # accumulate  -su (x) su/N  straight onto the Cuu psum
                    nc.tensor.matmul(
                        cps0[:, 0:C], su_row[0:1, 0:P], su_nrow[0:1, :],
                        start=False, stop=True,
                    )
                    nc.tensor.matmul(
                        cps1[:, 0:P], su_row[0:1, P:C], su_nrow[0:1, P:C],
                        start=False, stop=True,
                    )
                    # Ctilde = cps * (1/N)  (bf16 out)
                    nc.scalar.activation(
                        ct_bf[:, 0, :], cps0[:, 0:C], AF.Copy, scale=INV_N
                    )
                    nc.vector.tensor_scalar_mul(
                        ct_bf[:, 1, P:C], cps1[:, 0:P], INV_N
                    )
                    # missing block by symmetry
                    ctt = psm.tile([P, P], BF16, tag="ctt", name="ctt")
                    nc.tensor.transpose(ctt[:], ct_bf[:, 0, P:C], ident_bf[:])
                    nc.scalar.copy(ct_bf[:, 1, 0:P], ctt[:])

            # ---- phase 2: statistics / W_eff --------------------------
            with tc.tile_pool(name="sm", bufs=1) as sm:
                ak_bf = sm.tile([P, 2, HD], BF16)
                av_bf = sm.tile([P, 2, HD], BF16)
                mk_bf = sm.tile([P, 2, HD], BF16)
                mv_bf = sm.tile([P, 2, HD], BF16)
                with tc.tile_pool(name="psA", bufs=1, space="PSUM") as psA:
                    aps = {}
                    for nm, wT_t in (("k", wkT_bf), ("v", wvT_bf)):
                        for t in range(2):
                            ap_t = psA.tile(
                                [P, HD], F32, tag=f"a{nm}{t}", name=f"a{nm}{t}"
                            )
                            aps[(nm, t)] = ap_t
                            for tp in range(2):
                                nc.tensor.matmul(
                                    ap_t[:],
                                    ct_bf[:, tp, t * P:(t + 1) * P],
                                    wT_t[:, tp, :],
                                    start=(tp == 0),
                                    stop=(tp == 1),
                                )
                    # copies + second-moment integrands
                    for t in range(2):
                        nc.scalar.copy(ak_bf[:, t, :], aps[("k", t)][:])
                        nc.vector.tensor_copy(av_bf[:, t, :], aps[("v", t)][:])
                        nc.vector.tensor_mul(
                            mk_bf[:, t, :], aps[("k", t)][:], wkT_bf[:, t, :]
                        )
                        nc.vector.tensor_mul(
                            mv_bf[:, t, :], aps[("v", t)][:], wvT_bf[:, t, :]
                        )
                    # variances in column format: vv[:,0:4]=vark, [:,4:8]=varv
                    vv = psA.tile([P, 8], F32, tag="vv", name="vv")
                    for base, m_t in ((0, mk_bf), (4, mv_bf)):
                        for g in range(4):
                            for tp in range(2):
                                nc.tensor.matmul(
                                    vv[:, base + g:base + g + 1],
                                    m_t[:, tp, g * P:(g + 1) * P],
                                    ones_bf[:],
                                    start=(tp == 0),
                                    stop=(tp == 1),
                                )
                    nc.scalar.activation(
                        rk_col[:], vv[:, 0:4], AF.Sqrt, bias=eps_col[:, 0:1]
                    )
                    nc.scalar.activation(
                        rv_col[:], vv[:, 4:8], AF.Sqrt, bias=eps_col[:, 0:1]
                    )
                    nc.vector.reciprocal(rk_col[:], rk_col[:])
                    nc.vector.reciprocal(rv_col[:], rv_col[:])
                    # fold scales into Wq rows / Wo^T rows
                    for jp in range(4):
                        nc.vector.tensor_scalar_mul(
                            wq_s[:, jp, :], wq_bf[:, jp, :], rk_col[:, jp:jp + 1]
                        )
                        nc.scalar.activation(
                            wot_s[:, jp, :], woT_bf[:, jp, :], AF.Copy,
                            scale=rv_col[:, jp:jp + 1],
                        )

                # per head-pair: kv^T block, mask, then W_eff accumulation
                with tc.tile_pool(name="psP", bufs=1, space="PSUM") as psP:
                    wps2 = [
                        psP.tile([P, OUT], F32, tag=f"weff{t}", name=f"wps{t}")
                        for t in range(2)
                    ]
                    for jp in range(4):
                        sl = slice(jp * P, (jp + 1) * P)
                        sd = psP.tile([P, P], F32, tag="sd", bufs=2, name="sd")
                        for tp in range(2):
                            nc.tensor.matmul(
                                sd[:],
                                wvT_bf[:, tp, sl],
                                ak_bf[:, tp, sl],
                                start=(tp == 0),
                                stop=(tp == 1),
                            )
                        kv_bf = sm.tile([P, P], BF16, tag=f"kv{jp}", name=f"kv{jp}")
                        nc.vector.tensor_mul(kv_bf[:], sd[:], mask_bf[:])
                        bx = psP.tile([P, OUT], F32, tag="bx", bufs=2, name="bx")
                        nc.tensor.matmul(
                            bx[:], kv_bf[:], wot_s[:, jp, :], start=True, stop=True
                        )
                        bx_bf = sm.tile([P, OUT], BF16, tag=f"bxb{jp}", name=f"bxb{jp}")
                        if jp % 2 == 0:
                            nc.scalar.copy(bx_bf[:], bx[:])
                        else:
                            nc.vector.tensor_copy(bx_bf[:], bx[:])
                        for t in range(2):
                            nc.tensor.matmul(
                                wps2[t][:],
                                wq_s[:, jp, t * P:(t + 1) * P],
                                bx_bf[:],
                                start=(jp == 0),
                                stop=(jp == 3),
                            )
                    nc.scalar.copy(weff[:, 0, :], wps2[0][:])
                    nc.vector.tensor_copy(weff[:, 1, :], wps2[1][:])

            # ---- phase 3: out^T = W_eff^T u^T (bf16 PE stream) --------
            with (
                tc.tile_pool(name="opool", bufs=3) as opool,
                tc.tile_pool(name="pout", bufs=4, space="PSUM") as pout,
            ):
                for s in range(NGROUPS):
                    osb = opool.tile([P, 2, GROUP], BF16, tag="osb", name="osb")
                    for ob in range(2):
                        po = pout.tile([P, GROUP], F32, tag="po", name="po")
                        for t in range(2):
                            nc.tensor.matmul(
                                po[:],
                                weff[:, t, ob * P:(ob + 1) * P],
                                uT[:, t, s * GROUP:(s + 1) * GROUP],
                                start=(t == 0),
                                stop=(t == 1),
                            )
                        if ob == 0:
                            nc.vector.tensor_copy(osb[:, ob, :], po[:])
                        else:
                            nc.scalar.copy(osb[:, ob, :], po[:])
                    nc.sync.dma_start(
                        out_d.rearrange("(a p) n -> p a n", p=P)[
                            :, :, s * GROUP:(s + 1) * GROUP
                        ],
                        osb[:],
                    )

    nc.compile()
    return nc


_NC_CACHE = None


def _get_nc():
    global _NC_CACHE
    if _NC_CACHE is None:
        _NC_CACHE = build_nc()
    return _NC_CACHE


def make_in_maps(u_src, Wq, Wk, Wv, Wo):
    """Per-core input dicts. Core c = (batch c//2, half c%2); its own
    half of the grid axis is permuted to the front of u.  Everything is
    cast to bf16 host-side; weights are pre-transposed as the kernel
    expects them."""
    bf = ml_dtypes.bfloat16
    wq_b = np.ascontiguousarray(Wq.astype(bf))
    wkt_b = np.ascontiguousarray(Wk.T.astype(bf))
    wvt_b = np.ascontiguousarray(Wv.T.astype(bf))
    wot_b = np.ascontiguousarray(Wo.T.astype(bf))
    in_maps = []
    for c in range(8):
        b, half = c // 2, c % 2
        ub = u_src[b]
        mine = ub[half * N_HALF:(half + 1) * N_HALF]
        other = ub[(1 - half) * N_HALF:(2 - half) * N_HALF]
        u_perm = np.ascontiguousarray(
            np.concatenate([mine, other], axis=0).astype(bf)
        )
        in_maps.append(
            {"u": u_perm, "wq": wq_b, "wkt": wkt_b, "wvt": wvt_b, "wot": wot_b}
        )
    return in_maps


def assemble_output(results, bo):
    """Device emits out^T [OUT, N_HALF] bf16 with grid columns in
    (chunk, j, p) order; un-permute to rows (chunk, p, j) and upcast."""
    out = np.empty((4, N_FULL, OUT), dtype=np.float32)
    for c in range(8):
        b, half = c // 2, c % 2
        a = np.asarray(results[c]["out"]).astype(np.float32)  # [OUT, N_HALF]
        a = (
            a.reshape(OUT, N_CHUNKS // 2, SUBT, P)
            .transpose(1, 3, 2, 0)
            .reshape(N_HALF, OUT)
        )
        out[b, half * N_HALF:(half + 1) * N_HALF] = a
    if np.any(bo):
        out += bo.reshape(1, 1, OUT)
    return out


def run(inputs, trace=False, tmpdir=None):
    """inputs: dict as from reference.setup_inputs(). Returns
    (full_output, BassKernelResults)."""
    u_src = np.asarray(inputs["u_src"], dtype=np.float32)
    Wq = np.asarray(inputs["Wq"], dtype=np.float32)
    Wk = np.asarray(inputs["Wk"], dtype=np.float32)
    Wv = np.asarray(inputs["Wv"], dtype=np.float32)
    Wo = np.asarray(inputs["Wo"], dtype=np.float32)
    bo = np.asarray(inputs["bo"], dtype=np.float32)
    nc = _get_nc()
    in_maps = make_in_maps(u_src, Wq, Wk, Wv, Wo)
    res = run_bass_kernel_spmd(
        nc, in_maps, core_ids=list(range(8)), trace=trace, tmpdir=tmpdir
    )
    return assemble_output(res.results, bo), res


def kernel(**inputs):
    out, _ = run(inputs, trace=False)
    return out


# revision 12
# speedup vs baseline: 1.5349x; 1.0350x over previous
"""Trainium2 Bass kernel for nn_AttentionKernelIntegral (linear attention
with instance-normed k/v, collapsed algebraically).

Math
----
Reference computes (per batch, H=8 heads, D=64, C=OUT=256, N=16384):
    q = u @ Wq^T ; k = u @ Wk^T ; v = u @ Wv^T          (per head blocks)
    khat = instnorm_n(k); vhat = instnorm_n(v)
    kv_h = (1/N) khat_h^T vhat_h                        [D, D]
    out  = concat_h(q_h @ kv_h) @ Wo^T + bo

Everything downstream of u is linear except the instance-norm statistics
(exact functions of first/second moments over n), so the network
collapses to   out = u @ W_eff + bo.  With the *centered* covariance

    Ctilde = (Cuu - su su^T / N) / N,   Cuu = u^T u, su = u^T 1

the means drop out entirely:

    kv_h   = Dk_h (Wk_h Ctilde Wv_h^T) Dv_h
    vark_d = (Wk Ctilde Wk^T)_dd ;  Dk = diag(rsqrt(vark + eps))
    W_eff  = sum_h Wq_h^T kv_h Wo_h^T                   [C, OUT]

Sharding: 8 cores = 4 batches x 2 grid-halves.  Each core receives the
full u for its batch (bf16, with ITS half permuted first), accumulates
Cuu over the full grid, and emits out^T for its own half.

Layouts: the host pre-packs u / weights / output DRAM tensors
partition-major so every DMA descriptor moves 2-8 KB contiguous per
partition.  u and weights are bf16 (host cast); output is stored bf16
(out^T) and upcast + unpermuted on the host.

Cuu uses symmetry: the row-block-1 matmul streams only cols 128..256;
the missing [128,128] block of Ctilde is reconstructed by one PE
transpose.  The -su su^T/N correction is accumulated straight onto the
Cuu PSUM banks by two K=1 matmuls.  Variances are produced directly in
column format (N=1 matmuls against a ones column); rv is folded into
the per-pair block-diag mask and rk into the bx copy, so no scaled
weight copies are needed.
"""

import numpy as np
import ml_dtypes

import concourse.bass as bass
import concourse.tile as tile
from concourse import bacc, mybir
from concourse.bass_utils import run_bass_kernel_spmd
from concourse.masks import make_identity, make_block_diagonal

F32 = mybir.dt.float32
BF16 = mybir.dt.bfloat16
AL = mybir.AluOpType
AF = mybir.ActivationFunctionType

P = 128
N_FULL = 16384
N_HALF = 8192
C = 256
HD = 512          # H * D
OUT = 256
EPS = 1e-5
CH_ROWS = 2048
N_CHUNKS = N_FULL // CH_ROWS      # 8 chunks of 2048 rows (full grid)
SUBT = CH_ROWS // P               # 16 row-subtiles per chunk
G_ALL = N_FULL // P               # 128 row-tiles total
G_MINE = N_HALF // P              # first 64 belong to this core
INV_N = 1.0 / float(N_FULL)
GROUP = 512                       # phase-3 column group of out^T
NGROUPS = N_HALF // GROUP         # 16
OCH = 4                           # phase-3 store chunks (4 groups each)
U_ROW = N_CHUNKS * SUBT * C       # 32768 per-partition elements of u_r
O_ROW = OCH * 2 * CH_ROWS         # 16384 per-partition elements of out_r


def build_nc():
    nc = bacc.Bacc(
        "TRN2",
        target_bir_lowering=False,
        debug=False,
        num_devices=8,
    )
    u_d = nc.dram_tensor("u", [P, U_ROW], BF16, kind="ExternalInput").ap()
    wq_d = nc.dram_tensor("wq", [P, 4 * C], BF16, kind="ExternalInput").ap()
    wkv_d = nc.dram_tensor("wkv", [P, 2 * 2 * HD], BF16, kind="ExternalInput").ap()
    wot_d = nc.dram_tensor("wot", [P, 4 * OUT], BF16, kind="ExternalInput").ap()
    out_d = nc.dram_tensor("out", [P, O_ROW], BF16, kind="ExternalOutput").ap()

    with tile.TileContext(nc) as tc:
        with tc.tile_pool(name="pers", bufs=1) as pers:
            # ---- persistent tiles -------------------------------------
            uT = pers.tile([P, 2, N_HALF], BF16)         # u^T (bf16, own half)
            ident = pers.tile([P, P], F32)
            make_identity(nc, ident[:])
            ident_bf = pers.tile([P, P], BF16)
            nc.vector.tensor_copy(ident_bf[:], ident[:])
            mask_f = pers.tile([P, P], F32)
            make_block_diagonal(nc, mask_f[:], 64)
            mask_bf = pers.tile([P, P], BF16)
            nc.vector.tensor_copy(mask_bf[:], mask_f[:])
            wq_bf = pers.tile([P, 4, C], BF16)           # Wq natural [hd, c]
            wkvT_bf = pers.tile([P, 2, 2 * HD], BF16)    # [Wk^T | Wv^T] [c, 2hd]
            woT_bf = pers.tile([P, 4, OUT], BF16)        # Wo^T  [hd, o]
            weff = pers.tile([P, 2, OUT], BF16)
            ct_bf = pers.tile([P, 2, C], BF16)           # Ctilde (bf16)
            ones_bf = pers.tile([P, 1], BF16)
            nc.vector.memset(ones_bf[:], 1.0)
            eps_col = pers.tile([P, 1], F32)
            nc.vector.memset(eps_col[:], EPS)
            su_col = pers.tile([P, 2], BF16)
            su_row = pers.tile([1, C], BF16)
            su_nrow = pers.tile([1, C], BF16)            # -su / N
            rk_col = pers.tile([P, 4], F32)
            rv_col = pers.tile([P, 4], F32)
            mask_rv = pers.tile([P, 4, P], BF16)         # mask * rv (per pair)
            # prewarm ACT tables used later (Copy via scalar.mul, Sqrt)
            warm = pers.tile([1, 8], F32)
            nc.vector.memset(warm[:], 1.0)
            nc.scalar.mul(warm[:], warm[:], 1.0)
            nc.scalar.activation(warm[:], warm[:], AF.Sqrt)

            # ---- phase 1: stream u, accumulate Cuu, transpose own half
            with (
                tc.tile_pool(name="upool", bufs=3) as upool,
                tc.tile_pool(name="pacc", bufs=1, space="PSUM") as pacc,
                tc.tile_pool(name="ptr", bufs=3, space="PSUM") as ptr,
            ):
                cps0 = pacc.tile([P, C + 1], F32, tag="c0", name="c0")
                cps1 = pacc.tile([P, C + 1 - P], F32, tag="c1", name="c1")
                # chunk 0 arrives in three j-slices so the PE starts early
                sched = [(0, 0, 4), (0, 4, 4), (0, 8, 8)]
                for ch in range(1, N_CHUNKS):
                    sched.append((ch, 0, SUBT))
                ub = None
                first_dmas = 0
                for ch, j0, nsub in sched:
                    if j0 == 0:
                        ub = upool.tile([P, SUBT, C + 1], BF16, tag="ub", name="ub")
                    src_ap = u_d[:, ch * SUBT * C:(ch + 1) * SUBT * C].rearrange(
                        "p (j c) -> p j c", c=C
                    )
                    nc.sync.dma_start(
                        ub[:, j0:j0 + nsub, 0:C], src_ap[:, j0:j0 + nsub, :]
                    )
                    nc.vector.memset(ub[:, j0:j0 + nsub, C:C + 1], 1.0)
                    if first_dmas == 0:
                        nc.sync.dma_start(
                            wq_bf[:], wq_d.rearrange("p (a c) -> p a c", c=C)
                        )
                        nc.sync.dma_start(
                            wkvT_bf[:],
                            wkv_d.rearrange("p (a c) -> p a c", c=2 * HD),
                        )
                        nc.sync.dma_start(
                            woT_bf[:], wot_d.rearrange("p (a c) -> p a c", c=OUT)
                        )
                        first_dmas = 1
                    for j in range(j0, j0 + nsub):
                        g = ch * SUBT + j
                        nc.tensor.matmul(
                            cps0[:],
                            ub[:, j, 0:P],
                            ub[:, j, :],
                            start=(g == 0),
                            stop=(g == G_ALL - 1),
                        )
                        nc.tensor.matmul(
                            cps1[:],
                            ub[:, j, P:C],
                            ub[:, j, P:C + 1],
                            start=(g == 0),
                            stop=(g == G_ALL - 1),
                        )
                        if g < G_MINE:
                            tps = ptr.tile([P, C], BF16, tag="uT", name="tps")
                            for t in range(2):
                                nc.tensor.transpose(
                                    tps[:, t * P:(t + 1) * P],
                                    ub[:, j, t * P:(t + 1) * P],
                                    ident_bf[:],
                                )
                            if g % 2 == 0:
                                nc.vector.tensor_copy(
                                    uT[:, :, g * P:(g + 1) * P],
                                    tps[:].rearrange("p (t n) -> p t n", t=2),
                                )
                            else:
                                nc.scalar.copy(
                                    uT[:, :, g * P:(g + 1) * P],
                                    tps[:].rearrange("p (t n) -> p t n", t=2),
                                )

                # ---- Ctilde from cps + su (still holding cps psum) ----
                with tc.tile_pool(name="psm", bufs=1, space="PSUM") as psm:
                    nc.scalar.copy(su_col[:, 0:1], cps0[:, C:C + 1])
                    nc.vector.tensor_copy(su_col[:, 1:2], cps1[:, C - P:C - P + 1])
                    su_rowT = psm.tile([1, C], F32, tag="surt", name="surt")
                    for t in range(2):
                        nc.tensor.matmul(
                            su_rowT[0:1, t * P:(t + 1) * P],
                            su_col[:, t:t + 1],
                            ident_bf[:],
                            start=True,
                            stop=True,
                        )
                    nc.vector.tensor_copy(su_row[:], su_rowT[:])
                    nc.scalar.activation(
                        su_nrow[:], su_rowT[:], AF.Copy, scale=-INV_N
                    )
                    # accumulate  -su (x) su/N  straight onto the Cuu psum
                    nc.tensor.matmul(
                        cps0[:, 0:C], su_row[0:1, 0:P], su_nrow[0:1, :],
                        start=False, stop=True,
                    )
                    nc.tensor.matmul(
                        cps1[:, 0:P], su_row[0:1, P:C], su_nrow[0:1, P:C],
                        start=False, stop=True,
                    )
                    # Ctilde = cps * (1/N)  (bf16 out)
                    nc.scalar.activation(
                        ct_bf[:, 0, :], cps0[:, 0:C], AF.Copy, scale=INV_N
                    )
                    nc.vector.tensor_scalar_mul(
                        ct_bf[:, 1, P:C], cps1[:, 0:P], INV_N
                    )
                    # missing block by symmetry
                    ctt = psm.tile([P, P], BF16, tag="ctt", name="ctt")
                    nc.tensor.transpose(ctt[:], ct_bf[:, 0, P:C], ident_bf[:])
                    nc.vector.tensor_copy(ct_bf[:, 1, 0:P], ctt[:])

            # ---- phase 2: statistics / W_eff --------------------------
            with tc.tile_pool(name="sm", bufs=1) as sm:
                ak_bf = sm.tile([P, 2, HD], BF16)
                m_kv = sm.tile([P, 2, 2 * HD], BF16)
                with tc.tile_pool(name="psA", bufs=1, space="PSUM") as psA:
                    # a = Ctilde @ [Wk^T | Wv^T]   [c, 2hd]
                    aps = []
                    for t in range(2):
                        ap_t = psA.tile([P, 2 * HD], F32, tag=f"a{t}", name=f"a{t}")
                        aps.append(ap_t)
                        for half in range(2):
                            for tp in range(2):
                                nc.tensor.matmul(
                                    ap_t[:, half * HD:(half + 1) * HD],
                                    ct_bf[:, tp, t * P:(t + 1) * P],
                                    wkvT_bf[:, tp, half * HD:(half + 1) * HD],
                                    start=(tp == 0),
                                    stop=(tp == 1),
                                )
                    # k-half copies (ACT) for the per-pair sd matmuls
                    for t in range(2):
                        nc.scalar.copy(ak_bf[:, t, :], aps[t][:, 0:HD])
                    # integrands m = a .* wT: k halves first so the rk
                    # chain (needed by bx) completes early
                    vv = psA.tile([P, 8], F32, tag="vv", name="vv")
                    for half in range(2):
                        for t in range(2):
                            nc.vector.tensor_mul(
                                m_kv[:, t, half * HD:(half + 1) * HD],
                                aps[t][:, half * HD:(half + 1) * HD],
                                wkvT_bf[:, t, half * HD:(half + 1) * HD],
                            )
                        for g in range(4):
                            for tp in range(2):
                                nc.tensor.matmul(
                                    vv[:, 4 * half + g:4 * half + g + 1],
                                    m_kv[:, tp, half * HD + g * P:
                                         half * HD + (g + 1) * P],
                                    ones_bf[:],
                                    start=(tp == 0),
                                    stop=(tp == 1),
                                )
                    # keep-warm: cheap PE work while DVE/ACT chains run
                    dum = psA.tile([P, P], BF16, tag="dum", name="dum")
                    nc.tensor.transpose(dum[:], ident_bf[:], ident_bf[:])
                    nc.scalar.activation(
                        rk_col[:], vv[:, 0:4], AF.Sqrt, bias=eps_col[:, 0:1]
                    )
                    nc.vector.reciprocal(rk_col[:], rk_col[:])
                    nc.scalar.activation(
                        rv_col[:], vv[:, 4:8], AF.Sqrt, bias=eps_col[:, 0:1]
                    )
                    nc.vector.reciprocal(rv_col[:], rv_col[:])
                    # fold rv into the block-diag mask (per pair)
                    for jp in range(4):
                        nc.vector.tensor_scalar_mul(
                            mask_rv[:, jp, :], mask_bf[:], rv_col[:, jp:jp + 1]
                        )
                    nc.tensor.transpose(dum[:], ident_bf[:], ident_bf[:])

                # per head-pair: kv^T block, mask*rv, bx*rk, W_eff accum
                with tc.tile_pool(name="psP", bufs=1, space="PSUM") as psP:
                    wps2 = [
                        psP.tile([P, OUT], F32, tag=f"weff{t}", name=f"wps{t}")
                        for t in range(2)
                    ]
                    for jp in range(4):
                        sl = slice(jp * P, (jp + 1) * P)
                        sd = psP.tile([P, P], F32, tag="sd", bufs=2, name="sd")
                        for tp in range(2):
                            nc.tensor.matmul(
                                sd[:],
                                wkvT_bf[:, tp, HD + jp * P:HD + (jp + 1) * P],
                                ak_bf[:, tp, sl],
                                start=(tp == 0),
                                stop=(tp == 1),
                            )
                        kv_bf = sm.tile([P, P], BF16, tag=f"kv{jp}", name=f"kv{jp}")
                        nc.vector.tensor_mul(kv_bf[:], sd[:], mask_rv[:, jp, :])
                        bx = psP.tile([P, OUT], F32, tag="bx", bufs=2, name="bx")
                        nc.tensor.matmul(
                            bx[:], kv_bf[:], woT_bf[:, jp, :], start=True, stop=True
                        )
                        bx_bf = sm.tile([P, OUT], BF16, tag=f"bxb{jp}", name=f"bxb{jp}")
                        nc.scalar.activation(
                            bx_bf[:], bx[:], AF.Copy, scale=rk_col[:, jp:jp + 1]
                        )
                        for t in range(2):
                            nc.tensor.matmul(
                                wps2[t][:],
                                wq_bf[:, jp, t * P:(t + 1) * P],
                                bx_bf[:],
                                start=(jp == 0),
                                stop=(jp == 3),
                            )
                    nc.scalar.copy(weff[:, 0, :], wps2[0][:])
                    nc.vector.tensor_copy(weff[:, 1, :], wps2[1][:])

            # ---- phase 3: out^T = W_eff^T u^T (bf16 PE stream) --------
            with (
                tc.tile_pool(name="opool", bufs=2) as opool,
                tc.tile_pool(name="pout", bufs=4, space="PSUM") as pout,
            ):
                for och in range(OCH):
                    osb = opool.tile([P, 2, CH_ROWS], BF16, tag="osb", name="osb")
                    for sg in range(NGROUPS // OCH):
                        s = och * (NGROUPS // OCH) + sg
                        for ob in range(2):
                            po = pout.tile([P, GROUP], F32, tag="po", name="po")
                            for t in range(2):
                                nc.tensor.matmul(
                                    po[:],
                                    weff[:, t, ob * P:(ob + 1) * P],
                                    uT[:, t, s * GROUP:(s + 1) * GROUP],
                                    start=(t == 0),
                                    stop=(t == 1),
                                )
                            dst = osb[:, ob, sg * GROUP:(sg + 1) * GROUP]
                            if ob == 0:
                                nc.vector.tensor_copy(dst, po[:])
                            else:
                                nc.scalar.copy(dst, po[:])
                    nc.sync.dma_start(
                        out_d[:, och * 2 * CH_ROWS:(och + 1) * 2 * CH_ROWS]
                        .rearrange("p (a n) -> p a n", a=2),
                        osb[:],
                    )

    nc.compile()
    return nc


_NC_CACHE = None


def _get_nc():
    global _NC_CACHE
    if _NC_CACHE is None:
        _NC_CACHE = build_nc()
    return _NC_CACHE


def make_in_maps(u_src, Wq, Wk, Wv, Wo):
    """Per-core input dicts. Core c = (batch c//2, half c%2); its own
    half of the grid axis is permuted to the front of u.  Everything is
    cast to bf16 and packed partition-major host-side."""
    bf = ml_dtypes.bfloat16
    wq_b = np.ascontiguousarray(
        Wq.reshape(4, P, C).transpose(1, 0, 2).reshape(P, 4 * C).astype(bf)
    )
    wkv = np.concatenate([Wk.T, Wv.T], axis=1)           # [C, 2HD]
    wkv_b = np.ascontiguousarray(
        wkv.reshape(2, P, 2 * HD).transpose(1, 0, 2).reshape(P, 4 * HD).astype(bf)
    )
    wot_b = np.ascontiguousarray(
        Wo.T.reshape(4, P, OUT).transpose(1, 0, 2).reshape(P, 4 * OUT).astype(bf)
    )
    in_maps = []
    for c in range(8):
        b, half = c // 2, c % 2
        ub = u_src[b]
        mine = ub[half * N_HALF:(half + 1) * N_HALF]
        other = ub[(1 - half) * N_HALF:(2 - half) * N_HALF]
        u_perm = np.concatenate([mine, other], axis=0)   # [N_FULL, C]
        u_r = np.ascontiguousarray(
            u_perm.reshape(N_CHUNKS, P, SUBT, C)
            .transpose(1, 0, 2, 3)
            .reshape(P, U_ROW)
            .astype(bf)
        )
        in_maps.append({"u": u_r, "wq": wq_b, "wkv": wkv_b, "wot": wot_b})
    return in_maps


def assemble_output(results, bo):
    """Device emits out_r [P, OCH, 2, 2048] bf16 where element
    (p, och, a, j*128+pc) = out[row och*2048 + pc*16 + j, o=a*128+p]."""
    out = np.empty((4, N_FULL, OUT), dtype=np.float32)
    for c in range(8):
        b, half = c // 2, c % 2
        a = np.asarray(results[c]["out"]).astype(np.float32)
        a = a.reshape(P, OCH, 2, SUBT, P)        # [p, och, a, j, pc]
        a = a.transpose(1, 4, 3, 2, 0).reshape(N_HALF, OUT)
        out[b, half * N_HALF:(half + 1) * N_HALF] = a
    if np.any(bo):
        out += bo.reshape(1, 1, OUT)
    return out


def run(inputs, trace=False, tmpdir=None):
    """inputs: dict as from reference.setup_inputs(). Returns
    (full_output, BassKernelResults)."""
    u_src = np.asarray(inputs["u_src"], dtype=np.float32)
    Wq = np.asarray(inputs["Wq"], dtype=np.float32)
    Wk = np.asarray(inputs["Wk"], dtype=np.float32)
    Wv = np.asarray(inputs["Wv"], dtype=np.float32)
    Wo = np.asarray(inputs["Wo"], dtype=np.float32)
    bo = np.asarray(inputs["bo"], dtype=np.float32)
    nc = _get_nc()
    in_maps = make_in_maps(u_src, Wq, Wk, Wv, Wo)
    res = run_bass_kernel_spmd(
        nc, in_maps, core_ids=list(range(8)), trace=trace, tmpdir=tmpdir
    )
    return assemble_output(res.results, bo), res


def kernel(**inputs):
    out, _ = run(inputs, trace=False)
    return out


# revision 17
# speedup vs baseline: 1.6734x; 1.0902x over previous
"""Trainium2 Bass kernel for nn_AttentionKernelIntegral (linear attention
with instance-normed k/v, collapsed algebraically).

Math
----
Reference computes (per batch, H=8 heads, D=64, C=OUT=256, N=16384):
    q = u @ Wq^T ; k = u @ Wk^T ; v = u @ Wv^T          (per head blocks)
    khat = instnorm_n(k); vhat = instnorm_n(v)
    kv_h = (1/N) khat_h^T vhat_h                        [D, D]
    out  = concat_h(q_h @ kv_h) @ Wo^T + bo

Everything downstream of u is linear except the instance-norm statistics
(exact functions of first/second moments over n), so the network
collapses to   out = u @ W_eff + bo.  With the *centered* covariance

    Ctilde = (Cuu - su su^T / N) / N,   Cuu = u^T u, su = u^T 1

the means drop out entirely:

    kv_h   = Dk_h (Wk_h Ctilde Wv_h^T) Dv_h
    vark_d = (Wk Ctilde Wk^T)_dd ;  Dk = diag(rsqrt(vark + eps))
    W_eff  = sum_h Wq_h^T kv_h Wo_h^T                   [C, OUT]

Sharding: 8 cores = 4 batches x 2 grid-halves.  Each core receives the
full u for its batch (bf16, with ITS half permuted first), accumulates
Cuu over the full grid, and emits out^T for its own half.

Layouts: the host pre-packs u / weights / output DRAM tensors
partition-major so every DMA descriptor moves 2-8 KB contiguous per
partition.  u and weights are bf16 (host cast); output is stored bf16
(out^T) and upcast + unpermuted on the host.

Cuu uses symmetry: the row-block-1 matmul streams only cols 128..256;
the missing [128,128] block of Ctilde is reconstructed by one PE
transpose.  The -su su^T/N correction is accumulated straight onto the
Cuu PSUM banks by two K=1 matmuls.  Variances are produced directly in
column format (N=1 matmuls against a ones column); rv is folded into
the per-pair block-diag mask and rk into the bx copy, so no scaled
weight copies are needed.
"""

import numpy as np
import ml_dtypes

import concourse.bass as bass
import concourse.tile as tile
from concourse import bacc, mybir
from concourse.bass_utils import run_bass_kernel_spmd
from concourse.masks import make_identity, make_block_diagonal

F32 = mybir.dt.float32
BF16 = mybir.dt.bfloat16
AL = mybir.AluOpType
AF = mybir.ActivationFunctionType

P = 128
N_FULL = 16384
N_HALF = 8192
C = 256
HD = 512          # H * D
OUT = 256
EPS = 1e-5
CH_ROWS = 2048
N_CHUNKS = N_FULL // CH_ROWS      # 8 chunks of 2048 rows (full grid)
SUBT = CH_ROWS // P               # 16 row-subtiles per chunk
G_ALL = N_FULL // P               # 128 row-tiles total
G_MINE = N_HALF // P              # first 64 belong to this core
INV_N = 1.0 / float(N_FULL)
GROUP = 512                       # phase-3 column group of out^T
NGROUPS = N_HALF // GROUP         # 16
OCH = 4                           # phase-3 store chunks (4 groups each)
C1 = C + 1                        # u row + embedded 1.0 (ones column)
U_ROW = N_CHUNKS * SUBT * C1      # per-partition elements of u_r
O_ROW = OCH * 2 * CH_ROWS         # 16384 per-partition elements of out_r


def build_nc():
    nc = bacc.Bacc(
        "TRN2",
        target_bir_lowering=False,
        debug=False,
        num_devices=8,
    )
    u_d = nc.dram_tensor("u", [P, U_ROW], BF16, kind="ExternalInput").ap()
    wq_d = nc.dram_tensor("wq", [P, 4 * C], BF16, kind="ExternalInput").ap()
    wkv_d = nc.dram_tensor("wkv", [P, 2 * 2 * HD], BF16, kind="ExternalInput").ap()
    wot_d = nc.dram_tensor("wot", [P, 4 * OUT], BF16, kind="ExternalInput").ap()
    out_d = nc.dram_tensor("out", [P, O_ROW], BF16, kind="ExternalOutput").ap()

    with tile.TileContext(nc) as tc:
        with tc.tile_pool(name="pers", bufs=1) as pers:
            # ---- persistent tiles -------------------------------------
            uT = pers.tile([P, 2, N_HALF], BF16)         # u^T (bf16, own half)
            ident = pers.tile([P, P], F32)
            make_identity(nc, ident[:])
            ident_bf = pers.tile([P, P], BF16)
            nc.vector.tensor_copy(ident_bf[:], ident[:])
            mask_f = pers.tile([P, P], F32)
            make_block_diagonal(nc, mask_f[:], 64)
            mask_bf = pers.tile([P, P], BF16)
            nc.vector.tensor_copy(mask_bf[:], mask_f[:])
            wq_bf = pers.tile([P, 4, C], BF16)           # Wq natural [hd, c]
            wkvT_bf = pers.tile([P, 2, 2 * HD], BF16)    # [Wk^T | Wv^T] [c, 2hd]
            woT_bf = pers.tile([P, 4, OUT], BF16)        # Wo^T  [hd, o]
            weff = pers.tile([P, 2, OUT], BF16)
            ct_bf = pers.tile([P, 2, C], BF16)           # Ctilde (bf16)
            ones_bf = pers.tile([P, 1], BF16)
            nc.vector.memset(ones_bf[:], 1.0)
            eps_col = pers.tile([P, 1], F32)
            nc.vector.memset(eps_col[:], EPS)
            su_col = pers.tile([P, 2], BF16)
            su_row = pers.tile([1, C], BF16)
            su_nrow = pers.tile([1, C], BF16)            # -su / N
            rk_col = pers.tile([P, 4], F32)
            rv_col = pers.tile([P, 4], F32)
            mask_rv = pers.tile([P, 4, P], BF16)         # mask * rv (per pair)
            # prewarm ACT tables used later (Copy via scalar.mul, Sqrt)
            warm = pers.tile([1, 8], F32)
            nc.vector.memset(warm[:], 1.0)
            nc.scalar.mul(warm[:], warm[:], 1.0)
            nc.scalar.activation(warm[:], warm[:], AF.Sqrt)

            # ---- phase 1: stream u, accumulate Cuu, transpose own half
            with (
                tc.tile_pool(name="upool", bufs=3) as upool,
                tc.tile_pool(name="pacc", bufs=1, space="PSUM") as pacc,
                tc.tile_pool(name="ptr", bufs=3, space="PSUM") as ptr,
            ):
                cps0 = pacc.tile([P, C + 1], F32, tag="c0", name="c0")
                cps1 = pacc.tile([P, C + 1 - P], F32, tag="c1", name="c1")
                # chunk 0 arrives in three j-slices so the PE starts early
                sched = [(0, 0, 4), (0, 4, 4), (0, 8, 8)]
                for ch in range(1, N_CHUNKS):
                    sched.append((ch, 0, SUBT))
                ub = None
                first_dmas = 0
                for ch, j0, nsub in sched:
                    if j0 == 0:
                        ub = upool.tile([P, SUBT, C1], BF16, tag="ub", name="ub")
                    src_ap = u_d[:, ch * SUBT * C1:(ch + 1) * SUBT * C1].rearrange(
                        "p (j c) -> p j c", c=C1
                    )
                    nc.sync.dma_start(
                        ub[:, j0:j0 + nsub, :], src_ap[:, j0:j0 + nsub, :]
                    )
                    if first_dmas == 0:
                        nc.sync.dma_start(
                            wq_bf[:], wq_d.rearrange("p (a c) -> p a c", c=C)
                        )
                        nc.sync.dma_start(
                            wkvT_bf[:],
                            wkv_d.rearrange("p (a c) -> p a c", c=2 * HD),
                        )
                        nc.sync.dma_start(
                            woT_bf[:], wot_d.rearrange("p (a c) -> p a c", c=OUT)
                        )
                        first_dmas = 1
                    for j in range(j0, j0 + nsub):
                        g = ch * SUBT + j
                        nc.tensor.matmul(
                            cps0[:],
                            ub[:, j, 0:P],
                            ub[:, j, :],
                            start=(g == 0),
                            stop=(g == G_ALL - 1),
                        )
                        nc.tensor.matmul(
                            cps1[:],
                            ub[:, j, P:C],
                            ub[:, j, P:C + 1],
                            start=(g == 0),
                            stop=(g == G_ALL - 1),
                        )
                        if g < G_MINE:
                            tps = ptr.tile([P, C], BF16, tag="uT", name="tps")
                            for t in range(2):
                                nc.tensor.transpose(
                                    tps[:, t * P:(t + 1) * P],
                                    ub[:, j, t * P:(t + 1) * P],
                                    ident_bf[:],
                                )
                            if g % 2 == 0:
                                nc.vector.tensor_copy(
                                    uT[:, :, g * P:(g + 1) * P],
                                    tps[:].rearrange("p (t n) -> p t n", t=2),
                                )
                            else:
                                nc.scalar.copy(
                                    uT[:, :, g * P:(g + 1) * P],
                                    tps[:].rearrange("p (t n) -> p t n", t=2),
                                )

                # ---- Ctilde from cps + su (still holding cps psum) ----
                with tc.tile_pool(name="psm", bufs=1, space="PSUM") as psm:
                    nc.scalar.copy(su_col[:, 0:1], cps0[:, C:C + 1])
                    nc.vector.tensor_copy(su_col[:, 1:2], cps1[:, C - P:C - P + 1])
                    su_rowT = psm.tile([1, C], F32, tag="surt", name="surt")
                    for t in range(2):
                        nc.tensor.matmul(
                            su_rowT[0:1, t * P:(t + 1) * P],
                            su_col[:, t:t + 1],
                            ident_bf[:],
                            start=True,
                            stop=True,
                        )
                    nc.vector.tensor_copy(su_row[:], su_rowT[:])
                    nc.scalar.activation(
                        su_nrow[:], su_rowT[:], AF.Copy, scale=-INV_N
                    )
                    # accumulate  -su (x) su/N  straight onto the Cuu psum
                    nc.tensor.matmul(
                        cps0[:, 0:C], su_row[0:1, 0:P], su_nrow[0:1, :],
                        start=False, stop=True,
                    )
                    nc.tensor.matmul(
                        cps1[:, 0:P], su_row[0:1, P:C], su_nrow[0:1, P:C],
                        start=False, stop=True,
                    )
                    # Ctilde = cps * (1/N)  (bf16 out)
                    nc.scalar.activation(
                        ct_bf[:, 0, :], cps0[:, 0:C], AF.Copy, scale=INV_N
                    )
                    nc.vector.tensor_scalar_mul(
                        ct_bf[:, 1, P:C], cps1[:, 0:P], INV_N
                    )
                    # missing block by symmetry
                    ctt = psm.tile([P, P], BF16, tag="ctt", name="ctt")
                    nc.tensor.transpose(ctt[:], ct_bf[:, 0, P:C], ident_bf[:])
                    nc.vector.tensor_copy(ct_bf[:, 1, 0:P], ctt[:])

            # ---- phase 2: statistics / W_eff --------------------------
            with tc.tile_pool(name="sm", bufs=1) as sm:
                ak_bf = sm.tile([P, 2, HD], BF16)
                m_kv = sm.tile([P, 2, 2 * HD], BF16)
                with tc.tile_pool(name="psA", bufs=1, space="PSUM") as psA:
                    # a = Ctilde @ [Wk^T | Wv^T]   [c, 2hd]
                    aps = []
                    for t in range(2):
                        ap_t = psA.tile([P, 2 * HD], F32, tag=f"a{t}", name=f"a{t}")
                        aps.append(ap_t)
                        for half in range(2):
                            for tp in range(2):
                                nc.tensor.matmul(
                                    ap_t[:, half * HD:(half + 1) * HD],
                                    ct_bf[:, tp, t * P:(t + 1) * P],
                                    wkvT_bf[:, tp, half * HD:(half + 1) * HD],
                                    start=(tp == 0),
                                    stop=(tp == 1),
                                )
                    # v-chain first (mask_rv gates the per-pair kv muls):
                    # m_v from psum on DVE, while ACT copies the k halves
                    vv = psA.tile([P, 8], F32, tag="vv", name="vv")
                    for t in range(2):
                        nc.vector.tensor_mul(
                            m_kv[:, t, HD:2 * HD], aps[t][:, HD:2 * HD],
                            wkvT_bf[:, t, HD:2 * HD],
                        )
                        nc.scalar.copy(ak_bf[:, t, :], aps[t][:, 0:HD])
                    for g in range(4):
                        for tp in range(2):
                            nc.tensor.matmul(
                                vv[:, 4 + g:4 + g + 1],
                                m_kv[:, tp, HD + g * P:HD + (g + 1) * P],
                                ones_bf[:],
                                start=(tp == 0),
                                stop=(tp == 1),
                            )
                    nc.scalar.activation(
                        rv_col[:], vv[:, 4:8], AF.Sqrt, bias=eps_col[:, 0:1]
                    )
                    nc.vector.reciprocal(rv_col[:], rv_col[:])
                    for jp in range(4):
                        nc.vector.tensor_scalar_mul(
                            mask_rv[:, jp, :], mask_bf[:], rv_col[:, jp:jp + 1]
                        )
                    # k-chain: m_k from the bf16 copies (2x DVE rate)
                    for t in range(2):
                        nc.vector.tensor_mul(
                            m_kv[:, t, 0:HD], ak_bf[:, t, :], wkvT_bf[:, t, 0:HD]
                        )
                    for g in range(4):
                        for tp in range(2):
                            nc.tensor.matmul(
                                vv[:, g:g + 1],
                                m_kv[:, tp, g * P:(g + 1) * P],
                                ones_bf[:],
                                start=(tp == 0),
                                stop=(tp == 1),
                            )
                    nc.scalar.activation(
                        rk_col[:], vv[:, 0:4], AF.Sqrt, bias=eps_col[:, 0:1]
                    )
                    nc.vector.reciprocal(rk_col[:], rk_col[:])

                # per head-pair: kv^T block, mask*rv, bx*rk, W_eff accum
                with tc.tile_pool(name="psP", bufs=1, space="PSUM") as psP:
                    wps2 = [
                        psP.tile([P, OUT], F32, tag=f"weff{t}", name=f"wps{t}")
                        for t in range(2)
                    ]
                    for jp in range(4):
                        sl = slice(jp * P, (jp + 1) * P)
                        sd = psP.tile([P, P], F32, tag="sd", bufs=2, name="sd")
                        for tp in range(2):
                            nc.tensor.matmul(
                                sd[:],
                                wkvT_bf[:, tp, HD + jp * P:HD + (jp + 1) * P],
                                ak_bf[:, tp, sl],
                                start=(tp == 0),
                                stop=(tp == 1),
                            )
                        kv_bf = sm.tile([P, P], BF16, tag=f"kv{jp}", name=f"kv{jp}")
                        nc.vector.tensor_mul(kv_bf[:], sd[:], mask_rv[:, jp, :])
                        bx = psP.tile([P, OUT], F32, tag="bx", bufs=2, name="bx")
                        nc.tensor.matmul(
                            bx[:], kv_bf[:], woT_bf[:, jp, :], start=True, stop=True
                        )
                        bx_bf = sm.tile([P, OUT], BF16, tag=f"bxb{jp}", name=f"bxb{jp}")
                        nc.scalar.activation(
                            bx_bf[:], bx[:], AF.Copy, scale=rk_col[:, jp:jp + 1]
                        )
                        for t in range(2):
                            nc.tensor.matmul(
                                wps2[t][:],
                                wq_bf[:, jp, t * P:(t + 1) * P],
                                bx_bf[:],
                                start=(jp == 0),
                                stop=(jp == 3),
                            )
                    nc.scalar.copy(weff[:, 0, :], wps2[0][:])
                    nc.vector.tensor_copy(weff[:, 1, :], wps2[1][:])

            # ---- phase 3: out^T = W_eff^T u^T (bf16 PE stream) --------
            with (
                tc.tile_pool(name="opool", bufs=2) as opool,
                tc.tile_pool(name="pout", bufs=6, space="PSUM") as pout,
            ):
                for och in range(OCH):
                    osb = opool.tile([P, 2, CH_ROWS], BF16, tag="osb", name="osb")
                    for sg in range(NGROUPS // OCH):
                        s = och * (NGROUPS // OCH) + sg
                        for ob in range(2):
                            po = pout.tile([P, GROUP], F32, tag="po", name="po")
                            for t in range(2):
                                nc.tensor.matmul(
                                    po[:],
                                    weff[:, t, ob * P:(ob + 1) * P],
                                    uT[:, t, s * GROUP:(s + 1) * GROUP],
                                    start=(t == 0),
                                    stop=(t == 1),
                                )
                            dst = osb[:, ob, sg * GROUP:(sg + 1) * GROUP]
                            if ob == 0:
                                nc.vector.tensor_copy(dst, po[:])
                            else:
                                nc.scalar.copy(dst, po[:])
                    nc.sync.dma_start(
                        out_d[:, och * 2 * CH_ROWS:(och + 1) * 2 * CH_ROWS]
                        .rearrange("p (a n) -> p a n", a=2),
                        osb[:],
                    )

    nc.compile()
    return nc


_NC_CACHE = None


def _get_nc():
    global _NC_CACHE
    if _NC_CACHE is None:
        _NC_CACHE = build_nc()
    return _NC_CACHE


def make_in_maps(u_src, Wq, Wk, Wv, Wo):
    """Per-core input dicts. Core c = (batch c//2, half c%2); its own
    half of the grid axis is permuted to the front of u.  Everything is
    cast to bf16 and packed partition-major host-side."""
    bf = ml_dtypes.bfloat16
    wq_b = np.ascontiguousarray(
        Wq.reshape(4, P, C).transpose(1, 0, 2).reshape(P, 4 * C).astype(bf)
    )
    wkv = np.concatenate([Wk.T, Wv.T], axis=1)           # [C, 2HD]
    wkv_b = np.ascontiguousarray(
        wkv.reshape(2, P, 2 * HD).transpose(1, 0, 2).reshape(P, 4 * HD).astype(bf)
    )
    wot_b = np.ascontiguousarray(
        Wo.T.reshape(4, P, OUT).transpose(1, 0, 2).reshape(P, 4 * OUT).astype(bf)
    )
    in_maps = []
    for c in range(8):
        b, half = c // 2, c % 2
        ub = u_src[b]
        mine = ub[half * N_HALF:(half + 1) * N_HALF]
        other = ub[(1 - half) * N_HALF:(2 - half) * N_HALF]
        u_perm = np.concatenate([mine, other], axis=0)   # [N_FULL, C]
        u_r = np.empty((P, N_CHUNKS, SUBT, C1), dtype=bf)
        u_r[:, :, :, 0:C] = u_perm.reshape(N_CHUNKS, P, SUBT, C).transpose(
            1, 0, 2, 3
        ).astype(bf)
        u_r[:, :, :, C] = bf(1.0)
        u_r = u_r.reshape(P, U_ROW)
        in_maps.append({"u": u_r, "wq": wq_b, "wkv": wkv_b, "wot": wot_b})
    return in_maps


def assemble_output(results, bo):
    """Device emits out_r [P, OCH, 2, 2048] bf16 where element
    (p, och, a, j*128+pc) = out[row och*2048 + pc*16 + j, o=a*128+p]."""
    out = np.empty((4, N_FULL, OUT), dtype=np.float32)
    for c in range(8):
        b, half = c // 2, c % 2
        a = np.asarray(results[c]["out"]).astype(np.float32)
        a = a.reshape(P, OCH, 2, SUBT, P)        # [p, och, a, j, pc]
        a = a.transpose(1, 4, 3, 2, 0).reshape(N_HALF, OUT)
        out[b, half * N_HALF:(half + 1) * N_HALF] = a
    if np.any(bo):
        out += bo.reshape(1, 1, OUT)
    return out


def run(inputs, trace=False, tmpdir=None):
    """inputs: dict as from reference.setup_inputs(). Returns
    (full_output, BassKernelResults)."""
    u_src = np.asarray(inputs["u_src"], dtype=np.float32)
    Wq = np.asarray(inputs["Wq"], dtype=np.float32)
    Wk = np.asarray(inputs["Wk"], dtype=np.float32)
    Wv = np.asarray(inputs["Wv"], dtype=np.float32)
    Wo = np.asarray(inputs["Wo"], dtype=np.float32)
    bo = np.asarray(inputs["bo"], dtype=np.float32)
    nc = _get_nc()
    in_maps = make_in_maps(u_src, Wq, Wk, Wv, Wo)
    res = run_bass_kernel_spmd(
        nc, in_maps, core_ids=list(range(8)), trace=trace, tmpdir=tmpdir
    )
    return assemble_output(res.results, bo), res


def kernel(**inputs):
    out, _ = run(inputs, trace=False)
    return out
